# revision 1
# baseline (speedup 1.0000x reference)
"""TRN2 Bass kernel for nn_Attention_41506563948971.

Reference computation (per batch b):
    G  = (q @ w + b) @ a^T          [Lq, La]
    P  = softmax(G, axis=q)         (softmax over dim=1, the q axis)
    out= P^T @ q                    [La, H]

Sharding: data-parallel over batch B=8 across the 8 NeuronCores; w, b
replicated. Each core computes one full batch; no collectives.

Numerics: the logits G have sigma ~= 1024 (q,a ~ N(0,1), H=1024), so the
dim-q softmax is extremely peaked (top-2 gap ~ Exponential(mean 280)) and
logit errors translate directly into output errors on columns whose gap is
small. MM1/MM2 therefore run as 3-pass fp16 hi/lo split matmuls
(hi*hi + hi*lo + lo*hi ~= 22-bit operand precision, logit abs err ~2e-3;
bf16 splits give ~0.03 and 2-pass fp16 ~0.15, which measurably hurt
absmax). MM3's operands are one-hot-ish softmax weights and q, where
11-bit fp16 rounding gives ~2e-4 relative error at full 1-cycle/row PE
speed. The softmax normalization (1/sum) is folded into a per-partition
scale of the small MM3 output, so the big exp matrix is never divided.
All fp16/bf16/fp8-free matmuls run at 1 cycle/row on the PE; fp32 would
be 4x slower, and float32r (11-bit) matches fp16 anyway.

Schedule notes (cost-model span ~642 us/core, PE busy ~616 us at 96%
occupancy, i.e. at the 1-cycle/row matmul streaming floor):
- ~28 warmup matmuls fill the startup load-wait and pre-ramp the HAM
  clock gate so the real matmuls start at 2.4 GHz, not 1.2 GHz.
- q^T / a^T / E^T transposes go through the DMA xbar transpose engine
  (dma_start_transpose, out[p,k,j] = in[j, k*128+p]) on the ACT HWDGE
  queue, except the startup q-chunks where PE-transposes (batched 8 per
  PSUM bank, one strided DVE evacuation) avoid waiting on the load DMAs.
- bias-add, QwT hi-extract, and the output 1/sum scale run on the
  otherwise-idle ScalarE via Identity activations with AP bias/scale.
- MM2 runs nq-outer so each GT chunk's reduce_max overlaps the next
  chunk's matmuls; exps are emitted ahead of MM3's scales on ACT's
  in-order queue; MM3 is software-pipelined one a-tile behind so PE has
  work while ACT runs the exps.
"""

import sys

sys.path.insert(0, "/opt/trn_rl_repo")

from contextlib import ExitStack

import numpy as np

import concourse.bass as bass
import concourse.bacc as bacc
import concourse.mybir as mybir
import concourse.tile as tile
from concourse.masks import make_identity

dt = mybir.dt
AF = mybir.ActivationFunctionType
OP = mybir.AluOpType
AX = mybir.AxisListType

P = 128
H = 1024
KO = H // P          # 8 contraction chunks
LQ = 2048
LA = 2048
NQT = LQ // P        # 16 q row-tiles
NAT = LA // P        # 16 a row-tiles
QC = 512             # free-dim chunk (one fp32 PSUM bank)
NQC = LQ // QC       # 4
B = 8                # batch == number of cores

SPLIT_DT = dt.float16      # split format for MM1/MM2 hi/lo passes
SPLIT3 = ((0, 0), (0, 1), (1, 0))  # (hi,hi), (hi,lo), (lo,hi)
MM2_SPLITS = SPLIT3        # 2-pass loses too much logit precision (absmax)


def _split16(nc, pool, src_f32, tag):
    """Split an fp32 tile into (hi, lo) SPLIT_DT: hi = f16(x), lo = f16(x - hi).

    The subtract reads hi as fp16 directly (mixed-dtype tensor_tensor); the
    engine upconverts internally. hi+lo carries ~22 mantissa bits.
    """
    shape = list(src_f32.shape)
    hi = pool.tile(shape, SPLIT_DT, tag=f"{tag}_hi")
    lo = pool.tile(shape, SPLIT_DT, tag=f"{tag}_lo")
    nc.vector.tensor_copy(hi[:], src_f32[:])
    nc.vector.tensor_tensor(lo[:], src_f32[:], hi[:], OP.subtract)
    return hi, lo


def _trace_kernel(tc, q_d, a_d, w_d, b_d, o_d):
    nc = tc.nc
    with ExitStack() as ctx:
        pp = ctx.enter_context(tc.tile_pool(name="persist", bufs=1))
        # "scratch" serves the PE-transpose banks (phase-1 + a-tile 0),
        # the warmup, and MM3's output banks — their lifetimes never
        # overlap, so one 2-bank pool covers all three and frees a bank
        # for a 6th MM1/MM2 accumulator slot.
        ps_pool = ctx.enter_context(tc.tile_pool(name="ps", bufs=6, space="PSUM"))
        scratch = ctx.enter_context(tc.tile_pool(name="scratch", bufs=2, space="PSUM"))
        tp_pool = scratch
        op_pool = scratch

        id_sp = pp.tile([P, P], SPLIT_DT, tag="id_sp")
        make_identity(nc, id_sp[:])

        # PE clock warmup: the HAM gate holds the PE at 1.2 GHz until it
        # sees ~3.4 us of sustained activity, and the PE would otherwise
        # idle here waiting for the first q tile's load+split anyway.
        warm_sb = pp.tile([P, P], SPLIT_DT, tag="warm_sb")
        nc.vector.memset(warm_sb[:], 1.0)
        warm_ps = op_pool.tile([P, P], dt.float32, tag="tp", name="warm_ps")
        NWARM = 28
        for j in range(NWARM):
            nc.tensor.matmul(
                warm_ps[:], warm_sb[:], warm_sb[:],
                start=(j == 0), stop=(j == NWARM - 1),
            )

        b_sb = pp.tile([P, KO], dt.float32, tag="b_sb")

        # QwT = (q @ w + b)^T in [h, q] layout, stored as fp16 hi/lo splits.
        qwt_hi = pp.tile([P, KO, LQ], SPLIT_DT, tag="qwt_hi")
        qwt_lo = pp.tile([P, KO, LQ], SPLIT_DT, tag="qwt_lo")
        # q in natural [q, h] layout, rounded to fp16 for MM3.
        q_r = pp.tile([P, NQT, H], dt.float16, tag="q_r")

        # ---------------- Phase 1: MM1 -> QwT hi/lo ----------------
        with ExitStack() as p1:
            wpool = p1.enter_context(tc.tile_pool(name="wpool", bufs=1))
            stage = p1.enter_context(tc.tile_pool(name="stage", bufs=4))
            split = p1.enter_context(tc.tile_pool(name="split", bufs=2))
            qtp = p1.enter_context(tc.tile_pool(name="qtp", bufs=2))

            w_hi = wpool.tile([P, KO, H], SPLIT_DT, tag="w_hi")
            w_lo = wpool.tile([P, KO, H], SPLIT_DT, tag="w_lo")

            def load_w(k):
                wt = stage.tile([P, H], dt.float32, tag="wstage", name=f"wt{k}")
                nc.sync.dma_start(wt[:], w_d[k * P:(k + 1) * P, :])
                nc.vector.tensor_copy(w_hi[:, k], wt[:])
                nc.vector.tensor_tensor(w_lo[:, k], wt[:], w_hi[:, k], OP.subtract)

            def alloc_qt(qc):
                qt_hi = qtp.tile([P, KO, QC], SPLIT_DT, tag="qt_hi",
                                 name=f"qth{qc}")
                qt_lo = qtp.tile([P, KO, QC], SPLIT_DT, tag="qt_lo",
                                 name=f"qtl{qc}")
                return qt_hi, qt_lo

            def prep_q_tile(qc, t, qt, use_pe=False):
                qt_hi, qt_lo = qt
                qs = stage.tile([P, H], dt.float32, tag="qstage",
                                name=f"qs{qc}_{t}")
                row0 = qc * QC + t * P
                nc.sync.dma_start(qs[:], q_d[row0:row0 + P, :])
                qhi, qlo = _split16(nc, split, qs, "sp")
                nc.vector.tensor_copy(q_r[:, qc * (QC // P) + t], qs[:])
                if use_pe:
                    # PE transposes, batched 8 per PSUM bank with one
                    # strided DVE evacuation
                    for src, dst in ((qhi, qt_hi), (qlo, qt_lo)):
                        tp = tp_pool.tile([P, KO * P], SPLIT_DT, tag="tp")
                        for k in range(KO):
                            nc.tensor.transpose(
                                tp[:, k * P:(k + 1) * P],
                                src[:, k * P:(k + 1) * P],
                                id_sp[:],
                            )
                        nc.vector.tensor_copy(
                            dst[:, :, t * P:(t + 1) * P],
                            tp[:].rearrange("p (k c) -> p k c", k=KO),
                        )
                else:
                    # xbar DMA transpose (ACT HWDGE queue; loads on SP):
                    # out[p, k, j] = in[j, k*128 + p]
                    nc.scalar.dma_start_transpose(
                        qt_hi[:, :, t * P:(t + 1) * P], qhi[:])
                    nc.scalar.dma_start_transpose(
                        qt_lo[:, :, t * P:(t + 1) * P], qlo[:])

            # q-chunk 0's loads/splits/transposes first so PE starts
            # immediately; w loads overlap the transposes.
            qt_cur = alloc_qt(0)
            for t in range(QC // P):
                prep_q_tile(0, t, qt_cur, use_pe=True)
            # strided 1024-descriptor gather: keep it off the SP queue and
            # behind the startup-critical q loads
            nc.gpsimd.dma_start(b_sb[:], b_d.rearrange("(m p) -> p m", p=P))
            for k in range(KO):
                load_w(k)

            for qc in range(NQC):
                qt_hi, qt_lo = qt_cur
                if qc + 1 < NQC:
                    qt_next = alloc_qt(qc + 1)
                for m in range(KO):
                    acc = ps_pool.tile([P, QC], dt.float32, tag="ps")
                    n = 0
                    for wi, qi in SPLIT3:
                        lw = w_hi if wi == 0 else w_lo
                        rq = qt_hi if qi == 0 else qt_lo
                        for k in range(KO):
                            nc.tensor.matmul(
                                acc[:],
                                lw[:, k, m * P:(m + 1) * P],
                                rq[:, k, :],
                                start=(n == 0),
                                stop=(n == 3 * KO - 1),
                            )
                            n += 1
                    # bias add + hi-extract on ScalarE; only the lo subtract
                    # stays on VectorE (which is busy with q splits/evacs)
                    qwf = split.tile([P, QC], dt.float32, tag="qwf")
                    nc.scalar.activation(
                        qwf[:], acc[:], AF.Identity, bias=b_sb[:, m:m + 1]
                    )
                    dhi = qwt_hi[:, m, qc * QC:(qc + 1) * QC]
                    dlo = qwt_lo[:, m, qc * QC:(qc + 1) * QC]
                    nc.scalar.copy(dhi, qwf[:])
                    nc.vector.tensor_tensor(dlo, qwf[:], dhi, OP.subtract)
                    # interleave the next chunk's per-tile prep between
                    # m-blocks: DMA/DVE work lands just ahead of the PE
                    # transposes, so neither engine stalls
                    if qc + 1 < NQC and m < QC // P:
                        prep_q_tile(qc + 1, m, qt_next, use_pe=(qc == 0))
                if qc + 1 < NQC:
                    qt_cur = qt_next

        # ---------------- Phase 2: MM2 + softmax + MM3 ----------------
        with ExitStack() as p2:
            astage = p2.enter_context(tc.tile_pool(name="astage", bufs=4))
            asplit = p2.enter_context(tc.tile_pool(name="asplit", bufs=3))
            atp = p2.enter_context(tc.tile_pool(name="atp", bufs=2))
            ppool = p2.enter_context(tc.tile_pool(name="ppool", bufs=2))
            ptpool = p2.enter_context(tc.tile_pool(name="ptpool", bufs=2))
            outp = p2.enter_context(tc.tile_pool(name="outp", bufs=2))
            redp = p2.enter_context(tc.tile_pool(name="redp", bufs=4))

            def prep_a_tile(i, use_pe=False):
                at = astage.tile([P, H], dt.float32, tag="astage", name=f"at{i}")
                nc.sync.dma_start(at[:], a_d[i * P:(i + 1) * P, :])
                a_hi, a_lo = _split16(nc, asplit, at, "asp")
                at_hi = atp.tile([P, KO, P], SPLIT_DT, tag="at_hi", name=f"ath{i}")
                at_lo = atp.tile([P, KO, P], SPLIT_DT, tag="at_lo", name=f"atl{i}")
                if use_pe:
                    for src, dst in ((a_hi, at_hi), (a_lo, at_lo)):
                        tp = tp_pool.tile([P, KO * P], SPLIT_DT, tag="tp")
                        for k in range(KO):
                            nc.tensor.transpose(
                                tp[:, k * P:(k + 1) * P],
                                src[:, k * P:(k + 1) * P],
                                id_sp[:],
                            )
                        nc.vector.tensor_copy(
                            dst[:], tp[:].rearrange("p (k c) -> p k c", k=KO)
                        )
                else:
                    nc.scalar.dma_start_transpose(at_hi[:], a_hi[:])
                    nc.scalar.dma_start_transpose(at_lo[:], a_lo[:])
                return at_hi, at_lo

            def do_mm3(pt_sb, rinv, i):
                # MM3: out[a, h] = sum_q ET[q, a] * q[q, h], then * (1/sum)
                o_sb = outp.tile([P, H], dt.float32, tag="o_sb", name=f"osb{i}")
                for nh in range(H // QC):
                    acc = op_pool.tile([P, QC], dt.float32, tag="tp")
                    for t in range(NQT):
                        nc.tensor.matmul(
                            acc[:],
                            pt_sb[:, t, :],
                            q_r[:, t, nh * QC:(nh + 1) * QC],
                            start=(t == 0),
                            stop=(t == NQT - 1),
                        )
                    # 1/sum scale on ScalarE (Identity supports AP scale)
                    nc.scalar.activation(
                        o_sb[:, nh * QC:(nh + 1) * QC], acc[:], AF.Identity,
                        scale=rinv[:],
                    )
                nc.sync.dma_start(o_d[i * P:(i + 1) * P, :], o_sb[:])

            at_cur = prep_a_tile(0, use_pe=True)
            mm3_prev = None

            for i in range(NAT):
                at_hi, at_lo = at_cur

                # MM2 nq-outer: each GT chunk finishes early so its
                # reduce_max overlaps the next chunk's matmuls.
                gt = []
                gmax = redp.tile([P, NQC], dt.float32, tag="gmax")
                for nq in range(NQC):
                    # at the phase boundary (i==0) MM1's psum tiles are
                    # still draining; borrow the idle MM3 out-pool banks
                    # for the first chunks so MM2 starts immediately
                    g = ps_pool.tile([P, QC], dt.float32, tag="ps",
                                     name=f"gt{nq}")
                    n = 0
                    for ai, qi in MM2_SPLITS:
                        la_ = at_hi if ai == 0 else at_lo
                        rq = qwt_hi if qi == 0 else qwt_lo
                        for k in range(KO):
                            nc.tensor.matmul(
                                g[:],
                                la_[:, k, :],
                                rq[:, k, nq * QC:(nq + 1) * QC],
                                start=(n == 0),
                                stop=(n == len(MM2_SPLITS) * KO - 1),
                            )
                            n += 1
                    nc.vector.reduce_max(gmax[:, nq:nq + 1], g[:], axis=AX.X)
                    gt.append(g)

                negm = redp.tile([P, 1], dt.float32, tag="negm")
                nc.vector.reduce_max(negm[:], gmax[:], axis=AX.X, negate=True)

                # exps first so they're ahead of MM3's scales on ACT's
                # in-order queue
                p_sb = ppool.tile([P, LQ], dt.float16, tag="p_sb")
                sums = redp.tile([P, NQC], dt.float32, tag="sums")
                for nq in range(NQC):
                    nc.scalar.activation(
                        p_sb[:, nq * QC:(nq + 1) * QC],
                        gt[nq][:],
                        AF.Exp,
                        bias=negm[:],
                        scale=1.0,
                        accum_out=sums[:, nq:nq + 1],
                    )
                sall = redp.tile([P, 1], dt.float32, tag="sall")
                nc.vector.reduce_sum(sall[:], sums[:], axis=AX.X)
                rinv = redp.tile([P, 1], dt.float32, tag="rinv")
                nc.vector.reciprocal(rinv[:], sall[:])

                # PE work that needs no softmax results fills the window
                # while ACT runs the exps: next a-tile's transposes, then
                # the previous iteration's MM3.
                if i + 1 < NAT:
                    at_next = prep_a_tile(i + 1)
                if mm3_prev is not None:
                    do_mm3(*mm3_prev)

                # transpose E=[a,q] -> ET=[q,a] via xbar DMA, per chunk
                pt_sb = ptpool.tile([P, NQT, P], dt.float16, tag="pt_sb")
                for nq in range(NQC):
                    nc.scalar.dma_start_transpose(
                        pt_sb[:, nq * NQC:(nq + 1) * NQC, :],
                        p_sb[:, nq * QC:(nq + 1) * QC],
                    )

                mm3_prev = (pt_sb, rinv, i)
                if i + 1 < NAT:
                    at_cur = at_next

            do_mm3(*mm3_prev)


_CACHE = {}


def build_nc():
    if "nc" in _CACHE:
        return _CACHE["nc"]
    nc = bacc.Bacc("TRN2", target_bir_lowering=False, debug=False)
    q_d = nc.dram_tensor("q", [LQ, H], dt.float32, kind="ExternalInput").ap()
    a_d = nc.dram_tensor("a", [LA, H], dt.float32, kind="ExternalInput").ap()
    w_d = nc.dram_tensor("w", [H, H], dt.float32, kind="ExternalInput").ap()
    b_d = nc.dram_tensor("b", [H], dt.float32, kind="ExternalInput").ap()
    o_d = nc.dram_tensor("o", [LA, H], dt.float32, kind="ExternalOutput").ap()
    with tile.TileContext(nc) as tc:
        _trace_kernel(tc, q_d, a_d, w_d, b_d, o_d)
    nc.compile()
    _CACHE["nc"] = nc
    return nc


def get_runner():
    """Build (once) a cached jitted SPMD executable over the 8 cores.

    Mirrors bass2jax.run_bass_via_pjrt's multi-core path, but caches the
    jitted callable so repeated invocations don't recompile.
    """
    if "runner" in _CACHE:
        return _CACHE["runner"]
    import jax
    from jax.sharding import Mesh, PartitionSpec
    from jax.experimental.shard_map import shard_map

    from concourse import bass2jax

    nc = build_nc()
    bass2jax.install_neuronx_cc_hook()

    partition_name = nc.partition_id_tensor.name if nc.partition_id_tensor else None
    in_names, out_names, out_avals, zero_outs = [], [], [], []
    for alloc in nc.m.functions[0].allocations:
        if not isinstance(alloc, mybir.MemoryLocationSet):
            continue
        name = alloc.memorylocations[0].name
        if alloc.kind == "ExternalInput":
            if name != partition_name:
                in_names.append(name)
        elif alloc.kind == "ExternalOutput":
            shape = tuple(alloc.tensor_shape)
            dtype = mybir.dt.np(alloc.dtype)
            out_names.append(name)
            out_avals.append(jax.core.ShapedArray(shape, dtype))
            zero_outs.append(np.zeros(shape, dtype))
    n_params = len(in_names)
    all_in_names = list(in_names) + list(out_names)
    if partition_name is not None:
        all_in_names.append(partition_name)

    def _body(*args):
        operands = list(args)
        if partition_name is not None:
            operands.append(bass2jax.partition_id_tensor())
        outs = bass2jax._bass_exec_p.bind(
            *operands,
            out_avals=tuple(out_avals),
            in_names=tuple(all_in_names),
            out_names=tuple(out_names),
            lowering_input_output_aliases=(),
            sim_require_finite=True,
            sim_require_nnan=True,
            nc=nc,
        )
        return tuple(outs)

    devices = jax.devices()[:B]
    mesh = Mesh(np.asarray(devices), ("core",))
    n_outs = len(out_names)
    in_specs = (PartitionSpec("core"),) * (n_params + n_outs)
    out_specs = (PartitionSpec("core"),) * n_outs
    sharded = jax.jit(
        shard_map(
            _body, mesh=mesh, in_specs=in_specs, out_specs=out_specs, check_rep=False
        ),
        keep_unused=True,
    )
    runner = (sharded, in_names, out_names, out_avals, zero_outs)
    _CACHE["runner"] = runner
    return runner


def run_cores(in_maps):
    """Run the kernel SPMD over 8 cores; in_maps is a list of 8 dicts."""
    sharded, in_names, out_names, out_avals, zero_outs = get_runner()
    concat_in = [
        np.concatenate([np.asarray(m[name]) for m in in_maps], axis=0)
        for name in in_names
    ]
    concat_zeros = [
        np.zeros((B * z.shape[0], *z.shape[1:]), z.dtype) for z in zero_outs
    ]
    out_arrs = sharded(*concat_in, *concat_zeros)
    return [
        {
            name: np.asarray(out_arrs[j]).reshape(B, *out_avals[j].shape)[c]
            for j, name in enumerate(out_names)
        }
        for c in range(B)
    ]


def kernel(q, a, w, b):
    q = np.ascontiguousarray(np.asarray(q, dtype=np.float32))
    a = np.ascontiguousarray(np.asarray(a, dtype=np.float32))
    w = np.ascontiguousarray(np.asarray(w, dtype=np.float32))
    b = np.ascontiguousarray(np.asarray(b, dtype=np.float32))
    assert q.shape == (B, LQ, H) and a.shape == (B, LA, H)
    assert w.shape == (H, H) and b.shape == (H,)

    in_maps = [{"q": q[i], "a": a[i], "w": w, "b": b} for i in range(B)]
    try:
        from concourse.bass_utils import run_bass_kernel_spmd

        results = run_bass_kernel_spmd(
            build_nc(), in_maps, core_ids=list(range(B))
        ).results
    except Exception:
        # fallback: cached jitted shard_map runner (same execution path)
        results = run_cores(in_maps)
    return np.stack([results[i]["o"] for i in range(B)], axis=0)



# revision 2
# speedup vs baseline: 1.9661x; 1.9661x over previous
"""TRN2 Bass kernel for nn_Attention_41506563948971.

Reference computation (per batch b):
    G  = (q @ w + b) @ a^T          [Lq, La]
    P  = softmax(G, axis=q)         (softmax over dim=1, the q axis)
    out= P^T @ q                    [La, H]

Sharding: data-parallel over batch B=8 across the 8 NeuronCores; w, b
replicated. Each core computes one full batch; no collectives.

Numerics: logits G have sigma ~= 1024 (q,a ~ N(0,1), H=1024), so the dim-q
softmax is peaked and logit errors on near-max entries turn into output
errors. MM1/MM2 run as single-pass float32r matmuls: the PE reads 4-byte
fp32 operands rounded to ~FP22 (13-bit mantissa) at the full 1-cycle/row
rate (4x the true-fp32 rate), giving logit abs err ~5e-3 - no hi/lo split
passes needed. The BIR verifier requires every fp32r matmul operand to be
produced by an instruction that rounds to fp32r, so the w/qT/QwT/aT tiles
are written with float32r output dtype by the DVE/ACT ops that stage them.
MM3's operands are one-hot-ish softmax weights and q, where 11-bit fp16
rounding gives ~2e-4 relative error at full PE speed. The softmax
normalization (1/sum) is folded into a per-partition scale on the small
MM3 output, so the big exp matrix is never divided.

Schedule notes (cost-model span ~330 us/core, PE busy ~300 us):
- ~28 warmup matmuls pre-ramp the HAM clock gate so the real matmuls
  start at 2.4 GHz, not 1.2 GHz.
- q^T and a^T are produced by PE transposes (fp32, 2 cycles/row, batched
  8 per PSUM region) with a single strided DVE evacuation that also
  performs the fp32r rounding; the xbar DMA transpose engine only handles
  2-byte dtypes so it is reserved for the fp16 E^T transpose.
- bias-add + QwT evacuation runs on ScalarE as an Identity activation
  with AP bias and float32r output dtype.
- MM2 runs nq-outer so each GT chunk's reduce_max overlaps the next
  chunk's matmuls; exps are emitted ahead of MM3's scales on ACT's
  in-order queue; MM3 is software-pipelined one a-tile behind so PE has
  work while ACT runs the exps.
"""

import sys

sys.path.insert(0, "/opt/trn_rl_repo")

from contextlib import ExitStack

import numpy as np

import concourse.bass as bass
import concourse.bacc as bacc
import concourse.mybir as mybir
import concourse.tile as tile
from concourse.masks import make_identity

dt = mybir.dt
AF = mybir.ActivationFunctionType
OP = mybir.AluOpType
AX = mybir.AxisListType

P = 128
H = 1024
KO = H // P          # 8 contraction chunks
LQ = 2048
LA = 2048
NQT = LQ // P        # 16 q row-tiles
NAT = LA // P        # 16 a row-tiles
QC = 512             # free-dim chunk (one fp32 PSUM bank)
NQC = LQ // QC       # 4
B = 8                # batch == number of cores


def _trace_kernel(tc, q_d, a_d, w_d, b_d, o_d):
    nc = tc.nc
    with ExitStack() as ctx:
        pp = ctx.enter_context(tc.tile_pool(name="persist", bufs=1))

        id_f32 = pp.tile([P, P], dt.float32, tag="id_f32")
        make_identity(nc, id_f32[:])

        # PE clock warmup (HAM gate holds PE at 1.2 GHz until ~3.4 us of
        # sustained activity; PE would idle waiting for the first loads).
        warm_sb = pp.tile([P, P], dt.float16, tag="warm_sb")
        nc.vector.memset(warm_sb[:], 1.0)

        b_sb = pp.tile([P, KO], dt.float32, tag="b_sb")

        # QwT = (q @ w + b)^T in [h, q] layout, fp32r (PE reads at full rate).
        qwt_r = pp.tile([P, KO, LQ], dt.float32r, tag="qwt_r")
        # q in natural [q, h] layout, fp16 for MM3.
        q_r = pp.tile([P, NQT, H], dt.float16, tag="q_r")

        # ---------------- Phase 1: MM1 -> QwT ----------------
        with ExitStack() as p1:
            ps_pool = p1.enter_context(
                tc.tile_pool(name="ps1", bufs=4, space="PSUM"))
            tp_pool = p1.enter_context(
                tc.tile_pool(name="tp1", bufs=2, space="PSUM"))
            wpool = p1.enter_context(tc.tile_pool(name="wpool", bufs=1))
            stage = p1.enter_context(tc.tile_pool(name="stage", bufs=4))
            qtp = p1.enter_context(tc.tile_pool(name="qtp", bufs=2))

            warm_ps = tp_pool.tile([P, P], dt.float32, tag="tp",
                                   name="warm_ps")
            NWARM = 28
            for j in range(NWARM):
                nc.tensor.matmul(
                    warm_ps[:], warm_sb[:], warm_sb[:],
                    start=(j == 0), stop=(j == NWARM - 1),
                )

            w_r = wpool.tile([P, KO, H], dt.float32r, tag="w_r")

            def load_w(k):
                wt = stage.tile([P, H], dt.float32, tag="wstage", name=f"wt{k}")
                nc.sync.dma_start(wt[:], w_d[k * P:(k + 1) * P, :])
                nc.vector.tensor_copy(w_r[:, k], wt[:])

            def prep_q_tile(qc, t, qt_r):
                qs = stage.tile([P, H], dt.float32, tag="qstage",
                                name=f"qs{qc}_{t}")
                row0 = qc * QC + t * P
                nc.sync.dma_start(qs[:], q_d[row0:row0 + P, :])
                nc.vector.tensor_copy(q_r[:, qc * (QC // P) + t], qs[:])
                # PE transpose, batched 8 per PSUM region, one strided DVE
                # evacuation that also rounds to fp32r
                tp = tp_pool.tile([P, KO * P], dt.float32, tag="tp")
                for k in range(KO):
                    nc.tensor.transpose(
                        tp[:, k * P:(k + 1) * P],
                        qs[:, k * P:(k + 1) * P],
                        id_f32[:],
                    )
                nc.vector.tensor_copy(
                    qt_r[:, :, t * P:(t + 1) * P],
                    tp[:].rearrange("p (k c) -> p k c", k=KO),
                )

            def alloc_qt(qc):
                return qtp.tile([P, KO, QC], dt.float32r, tag="qt_r",
                                name=f"qt{qc}")

            # q-chunk 0's loads/transposes first so PE starts immediately;
            # w loads overlap the transposes.
            qt_cur = alloc_qt(0)
            for t in range(QC // P):
                prep_q_tile(0, t, qt_cur)
            # strided 1024-descriptor gather: keep it off the SP queue and
            # behind the startup-critical q loads
            nc.gpsimd.dma_start(b_sb[:], b_d.rearrange("(m p) -> p m", p=P))
            for k in range(KO):
                load_w(k)

            for qc in range(NQC):
                if qc + 1 < NQC:
                    qt_next = alloc_qt(qc + 1)
                for m in range(KO):
                    acc = ps_pool.tile([P, QC], dt.float32, tag="ps")
                    for k in range(KO):
                        nc.tensor.matmul(
                            acc[:],
                            w_r[:, k, m * P:(m + 1) * P],
                            qt_cur[:, k, :],
                            start=(k == 0),
                            stop=(k == KO - 1),
                        )
                    # bias add + fp32r rounding + evacuation on ScalarE
                    nc.scalar.activation(
                        qwt_r[:, m, qc * QC:(qc + 1) * QC], acc[:],
                        AF.Identity, bias=b_sb[:, m:m + 1],
                    )
                    # interleave the next chunk's per-tile prep between
                    # m-blocks so loads/transposes land just ahead of use
                    if qc + 1 < NQC and m < QC // P:
                        prep_q_tile(qc + 1, m, qt_next)
                if qc + 1 < NQC:
                    qt_cur = qt_next

        # ---------------- Phase 2: MM2 + softmax + MM3 ----------------
        with ExitStack() as p2:
            ps_pool = p2.enter_context(
                tc.tile_pool(name="ps2", bufs=6, space="PSUM"))
            tp_pool = p2.enter_context(
                tc.tile_pool(name="tp2", bufs=1, space="PSUM"))
            astage = p2.enter_context(tc.tile_pool(name="astage", bufs=3))
            atp = p2.enter_context(tc.tile_pool(name="atp", bufs=2))
            ppool = p2.enter_context(tc.tile_pool(name="ppool", bufs=2))
            ptpool = p2.enter_context(tc.tile_pool(name="ptpool", bufs=2))
            outp = p2.enter_context(tc.tile_pool(name="outp", bufs=2))
            redp = p2.enter_context(tc.tile_pool(name="redp", bufs=4))

            def prep_a_tile(i):
                at = astage.tile([P, H], dt.float32, tag="astage",
                                 name=f"at{i}")
                nc.sync.dma_start(at[:], a_d[i * P:(i + 1) * P, :])
                at_r = atp.tile([P, KO, P], dt.float32r, tag="at_r",
                                name=f"atr{i}")
                tp = tp_pool.tile([P, KO * P], dt.float32, tag="tp")
                for k in range(KO):
                    nc.tensor.transpose(
                        tp[:, k * P:(k + 1) * P],
                        at[:, k * P:(k + 1) * P],
                        id_f32[:],
                    )
                nc.vector.tensor_copy(
                    at_r[:], tp[:].rearrange("p (k c) -> p k c", k=KO)
                )
                return at_r

            def do_mm3(pt_sb, rinv, i):
                # MM3: out[a, h] = sum_q ET[q, a] * q[q, h], then * (1/sum)
                o_sb = outp.tile([P, H], dt.float32, tag="o_sb", name=f"osb{i}")
                for nh in range(H // QC):
                    acc = ps_pool.tile([P, QC], dt.float32, tag="ps",
                                       name=f"m3_{i}_{nh}")
                    for t in range(NQT):
                        nc.tensor.matmul(
                            acc[:],
                            pt_sb[:, t, :],
                            q_r[:, t, nh * QC:(nh + 1) * QC],
                            start=(t == 0),
                            stop=(t == NQT - 1),
                        )
                    # 1/sum scale on ScalarE (Identity supports AP scale)
                    nc.scalar.activation(
                        o_sb[:, nh * QC:(nh + 1) * QC], acc[:], AF.Identity,
                        scale=rinv[:],
                    )
                nc.sync.dma_start(o_d[i * P:(i + 1) * P, :], o_sb[:])

            at_cur = prep_a_tile(0)
            mm3_prev = None

            for i in range(NAT):
                # MM2 nq-outer: each GT chunk finishes early so its
                # reduce_max overlaps the next chunk's matmuls.
                gt = []
                gmax = redp.tile([P, NQC], dt.float32, tag="gmax")
                for nq in range(NQC):
                    g = ps_pool.tile([P, QC], dt.float32, tag="ps",
                                     name=f"gt{nq}")
                    for k in range(KO):
                        nc.tensor.matmul(
                            g[:],
                            at_cur[:, k, :],
                            qwt_r[:, k, nq * QC:(nq + 1) * QC],
                            start=(k == 0),
                            stop=(k == KO - 1),
                        )
                    nc.vector.reduce_max(gmax[:, nq:nq + 1], g[:], axis=AX.X)
                    gt.append(g)

                negm = redp.tile([P, 1], dt.float32, tag="negm")
                nc.vector.reduce_max(negm[:], gmax[:], axis=AX.X, negate=True)

                # exps first so they're ahead of MM3's scales on ACT's
                # in-order queue
                p_sb = ppool.tile([P, LQ], dt.float16, tag="p_sb")
                sums = redp.tile([P, NQC], dt.float32, tag="sums")
                for nq in range(NQC):
                    nc.scalar.activation(
                        p_sb[:, nq * QC:(nq + 1) * QC],
                        gt[nq][:],
                        AF.Exp,
                        bias=negm[:],
                        scale=1.0,
                        accum_out=sums[:, nq:nq + 1],
                    )
                sall = redp.tile([P, 1], dt.float32, tag="sall")
                nc.vector.reduce_sum(sall[:], sums[:], axis=AX.X)
                rinv = redp.tile([P, 1], dt.float32, tag="rinv")
                nc.vector.reciprocal(rinv[:], sall[:])

                # PE work that needs no softmax results fills the window
                # while ACT runs the exps: next a-tile's transposes, then
                # the previous iteration's MM3.
                if i + 1 < NAT:
                    at_next = prep_a_tile(i + 1)
                if mm3_prev is not None:
                    do_mm3(*mm3_prev)

                # transpose E=[a,q] -> ET=[q,a] via xbar DMA, per chunk
                pt_sb = ptpool.tile([P, NQT, P], dt.float16, tag="pt_sb")
                for nq in range(NQC):
                    nc.scalar.dma_start_transpose(
                        pt_sb[:, nq * NQC:(nq + 1) * NQC, :],
                        p_sb[:, nq * QC:(nq + 1) * QC],
                    )

                mm3_prev = (pt_sb, rinv, i)
                if i + 1 < NAT:
                    at_cur = at_next

            do_mm3(*mm3_prev)


_CACHE = {}


def build_nc():
    if "nc" in _CACHE:
        return _CACHE["nc"]
    nc = bacc.Bacc("TRN2", target_bir_lowering=False, debug=False)
    q_d = nc.dram_tensor("q", [LQ, H], dt.float32, kind="ExternalInput").ap()
    a_d = nc.dram_tensor("a", [LA, H], dt.float32, kind="ExternalInput").ap()
    w_d = nc.dram_tensor("w", [H, H], dt.float32, kind="ExternalInput").ap()
    b_d = nc.dram_tensor("b", [H], dt.float32, kind="ExternalInput").ap()
    o_d = nc.dram_tensor("o", [LA, H], dt.float32, kind="ExternalOutput").ap()
    with tile.TileContext(nc) as tc:
        _trace_kernel(tc, q_d, a_d, w_d, b_d, o_d)
    nc.compile()
    _CACHE["nc"] = nc
    return nc


def get_runner():
    """Build (once) a cached jitted SPMD executable over the 8 cores.

    Mirrors bass2jax.run_bass_via_pjrt's multi-core path, but caches the
    jitted callable so repeated invocations don't recompile.
    """
    if "runner" in _CACHE:
        return _CACHE["runner"]
    import jax
    from jax.sharding import Mesh, PartitionSpec
    from jax.experimental.shard_map import shard_map

    from concourse import bass2jax

    nc = build_nc()
    bass2jax.install_neuronx_cc_hook()

    partition_name = nc.partition_id_tensor.name if nc.partition_id_tensor else None
    in_names, out_names, out_avals, zero_outs = [], [], [], []
    for alloc in nc.m.functions[0].allocations:
        if not isinstance(alloc, mybir.MemoryLocationSet):
            continue
        name = alloc.memorylocations[0].name
        if alloc.kind == "ExternalInput":
            if name != partition_name:
                in_names.append(name)
        elif alloc.kind == "ExternalOutput":
            shape = tuple(alloc.tensor_shape)
            dtype = mybir.dt.np(alloc.dtype)
            out_names.append(name)
            out_avals.append(jax.core.ShapedArray(shape, dtype))
            zero_outs.append(np.zeros(shape, dtype))
    n_params = len(in_names)
    all_in_names = list(in_names) + list(out_names)
    if partition_name is not None:
        all_in_names.append(partition_name)

    def _body(*args):
        operands = list(args)
        if partition_name is not None:
            operands.append(bass2jax.partition_id_tensor())
        outs = bass2jax._bass_exec_p.bind(
            *operands,
            out_avals=tuple(out_avals),
            in_names=tuple(all_in_names),
            out_names=tuple(out_names),
            lowering_input_output_aliases=(),
            sim_require_finite=True,
            sim_require_nnan=True,
            nc=nc,
        )
        return tuple(outs)

    devices = jax.devices()[:B]
    mesh = Mesh(np.asarray(devices), ("core",))
    n_outs = len(out_names)
    in_specs = (PartitionSpec("core"),) * (n_params + n_outs)
    out_specs = (PartitionSpec("core"),) * n_outs
    sharded = jax.jit(
        shard_map(
            _body, mesh=mesh, in_specs=in_specs, out_specs=out_specs, check_rep=False
        ),
        keep_unused=True,
    )
    runner = (sharded, in_names, out_names, out_avals, zero_outs)
    _CACHE["runner"] = runner
    return runner


def run_cores(in_maps):
    """Run the kernel SPMD over 8 cores; in_maps is a list of 8 dicts."""
    sharded, in_names, out_names, out_avals, zero_outs = get_runner()
    concat_in = [
        np.concatenate([np.asarray(m[name]) for m in in_maps], axis=0)
        for name in in_names
    ]
    concat_zeros = [
        np.zeros((B * z.shape[0], *z.shape[1:]), z.dtype) for z in zero_outs
    ]
    out_arrs = sharded(*concat_in, *concat_zeros)
    return [
        {
            name: np.asarray(out_arrs[j]).reshape(B, *out_avals[j].shape)[c]
            for j, name in enumerate(out_names)
        }
        for c in range(B)
    ]


def kernel(q, a, w, b):
    q = np.ascontiguousarray(np.asarray(q, dtype=np.float32))
    a = np.ascontiguousarray(np.asarray(a, dtype=np.float32))
    w = np.ascontiguousarray(np.asarray(w, dtype=np.float32))
    b = np.ascontiguousarray(np.asarray(b, dtype=np.float32))
    assert q.shape == (B, LQ, H) and a.shape == (B, LA, H)
    assert w.shape == (H, H) and b.shape == (H,)

    in_maps = [{"q": q[i], "a": a[i], "w": w, "b": b} for i in range(B)]
    try:
        from concourse.bass_utils import run_bass_kernel_spmd

        results = run_bass_kernel_spmd(
            build_nc(), in_maps, core_ids=list(range(B))
        ).results
    except Exception:
        # fallback: cached jitted shard_map runner (same execution path)
        results = run_cores(in_maps)
    return np.stack([results[i]["o"] for i in range(B)], axis=0)
